# revision 32
# baseline (speedup 1.0000x reference)
"""Trainium2 Bass kernel for nn_DevelopmentalLayerV51 (moe_routing).

kernel(**inputs) takes the FULL unsharded inputs (as reference.setup_inputs)
and returns the full (h, dh) tuple of np.float32 arrays.

Sharding: data-parallel over the B*S=8192 tokens across 8 cores (core c owns
batch c//2, sequence half c%2 -> 1024 tokens). Top-k routing uses only
scores[0], so it is decided on host; only the selected blocks' weights are
shipped (replicated, bf16). On device all activations are feature-major
[D, tokens] so every matmul is transpose-free (contraction dim in
partitions for both operands); LayerNorm/softmax partition-reductions use
ones-matmuls on the PE. Attention K/V are AllGathered within the core pair
sharing a batch; delayed = mean_S(disembodied) via a tiny pair AllReduce.
Residual streams stay fp32 and round-trip through DRAM between phases so
SBUF only ever holds one phase's working set.
"""
import sys

sys.path.insert(0, "/opt/trn_rl_repo")

import contextlib

import numpy as np
import ml_dtypes

import concourse.bass as bass
import concourse.tile as tile
from concourse import bacc, mybir

DIM = 1024
NBLK = 8
NHEAD = 4
HD = DIM // NHEAD          # 256
B, S = 4, 2048
NCORES = 8
TLOC = (B * S) // NCORES   # 1024 tokens per core
P = 128
DC = DIM // P              # 8 feature tiles
DT = TLOC // P             # 8 token tiles
FF = 4 * DIM               # 4096
FC = FF // P               # 32
F32 = mybir.dt.float32
BF16 = mybir.dt.bfloat16
AF = mybir.ActivationFunctionType
AX = mybir.AxisListType
ALU = mybir.AluOpType

PAIRS = [[0, 1], [2, 3], [4, 5], [6, 7]]
QS = [slice(0, 512), slice(512, 1024)]


def build_program(kk):
    nc = bacc.Bacc("TRN2", target_bir_lowering=False, debug=False,
                   num_devices=NCORES)
    dp = nc.declare_dram_parameter

    x_emb = dp("x_emb", [DIM, TLOC], F32, isOutput=False)
    x_dis = dp("x_dis", [DIM, TLOC], F32, isOutput=False)
    common = dp("common", [P, DC, 2], F32, isOutput=False)  # esc, s05
    e_w1 = [dp(f"e{i}_w1", [DIM, DIM], BF16, isOutput=False) for i in range(kk)]
    e_w2 = [dp(f"e{i}_w2", [DIM, DIM], BF16, isOutput=False) for i in range(kk)]
    # cols: ln_s, ln_b, b1, esc*b2
    e_vec = [dp(f"e{i}_vec", [P, DC, 4], F32, isOutput=False) for i in range(kk)]
    d_wqk = [dp(f"d{i}_wqk", [DIM, 2 * DIM], BF16, isOutput=False) for i in range(kk)]
    d_wv = [dp(f"d{i}_wv", [DIM, DIM], BF16, isOutput=False) for i in range(kk)]
    d_wout = [dp(f"d{i}_wout", [DIM, DIM], BF16, isOutput=False) for i in range(kk)]
    d_ff1 = [dp(f"d{i}_ff1", [DIM, FF], BF16, isOutput=False) for i in range(kk)]
    d_ff2 = [dp(f"d{i}_ff2", [FF, DIM], BF16, isOutput=False) for i in range(kk)]
    # cols: ln1_s, ln1_b, ln2_s, ln2_b, bq/16, bk, b_out, 0.5*b_ff2
    d_vec = [dp(f"d{i}_vec", [P, DC, 8], F32, isOutput=False) for i in range(kk)]
    d_ff1b = [dp(f"d{i}_ff1b", [P, FC], F32, isOutput=False) for i in range(kk)]
    d_bv = [dp(f"d{i}_bv", [1, DIM], BF16, isOutput=False) for i in range(kk)]
    out_e = dp("out_e", [DIM, TLOC], F32, isOutput=True)
    out_d = dp("out_d", [DIM, TLOC], F32, isOutput=True)

    dt_ = nc.dram_tensor
    cc_del_in = dt_("cc_del_in", [P, DC], F32)
    cc_del_out = dt_("cc_del_out", [P, DC], F32)
    e_mid = [[dt_(f"e_mid{i}_{m}", [P, TLOC], F32) for m in range(DC)]
             for i in range(max(kk - 1, 0))]
    d_mid = [[dt_(f"d_mid{i}_{m}", [P, TLOC], F32) for m in range(DC)]
             for i in range(max(kk - 1, 0))]
    k_loc = [dt_(f"k_loc{i}", [DIM, TLOC], BF16) for i in range(kk)]
    v_loc = [dt_(f"v_loc{i}", [DIM, TLOC], BF16) for i in range(kk)]
    k_full = [dt_(f"k_full{i}", [2, DIM, TLOC], BF16) for i in range(kk)]
    v_full = [dt_(f"v_full{i}", [2, DIM, TLOC], BF16) for i in range(kk)]

    def res_ap(t, m):
        return t[m][:, :] if isinstance(t, list) else t[m * P:(m + 1) * P, :]

    e_src = [x_emb] + e_mid
    e_dst = e_mid + [out_e]
    d_src = [x_dis] + d_mid
    d_dst = d_mid + [out_d]

    with tile.TileContext(nc, pool_alloc_mode="queue") as tc, \
         contextlib.ExitStack() as octx:
        singles = octx.enter_context(tc.tile_pool(name="singles", bufs=1))
        stats = octx.enter_context(tc.tile_pool(name="stats", bufs=1))
        qpool = octx.enter_context(tc.tile_pool(name="qp", bufs=1))
        espan = octx.enter_context(tc.tile_pool(name="esp", bufs=1))
        psum = octx.enter_context(tc.tile_pool(name="psum", bufs=8,
                                               space="PSUM"))

        def ps_tile(name):
            return psum.tile([P, 512], F32, tag="ps", name=name)

        def ps1_tile(name):
            return psum.tile([1, 512], F32, tag="ps", name=name)

        ones128_bf = singles.tile([P, 1], BF16)
        nc.vector.memset(ones128_bf, 1.0)
        ones1_f32 = singles.tile([1, P], F32)
        nc.vector.memset(ones1_f32, 1.0)
        ones1_bf = singles.tile([1, P], BF16)
        nc.vector.memset(ones1_bf, 1.0)

        eps_sb = singles.tile([P, 1], F32)
        nc.vector.memset(eps_sb, 1e-5)
        common_sb = singles.tile([P, DC, 2], F32)
        nc.sync.dma_start(out=common_sb, in_=common[:, :, :])
        evec_sb = []
        dvec_sb = []
        ff1b_sb = []
        bv_sb = []
        for i in range(kk):
            t = singles.tile([P, DC, 4], F32, name=f"evec{i}", tag=f"evec{i}")
            nc.sync.dma_start(out=t, in_=e_vec[i][:, :, :])
            evec_sb.append(t)
            t = singles.tile([P, DC, 8], F32, name=f"dvec{i}", tag=f"dvec{i}")
            nc.sync.dma_start(out=t, in_=d_vec[i][:, :, :])
            dvec_sb.append(t)
            t = singles.tile([P, FC], F32, name=f"ff1b{i}", tag=f"ff1b{i}")
            nc.sync.dma_start(out=t, in_=d_ff1b[i][:, :])
            ff1b_sb.append(t)
            t = singles.tile([1, DIM], BF16, name=f"bv{i}", tag=f"bv{i}")
            nc.sync.dma_start(out=t, in_=d_bv[i][:, :])
            bv_sb.append(t)

        def act(out, in_, func=AF.Copy, bias=0.0, scale=1.0):
            if func == AF.Copy and not isinstance(bias, float):
                func = AF.Identity  # Copy rejects AP bias; Identity is affine
            nc.scalar.activation(out=out, in_=in_, func=func, bias=bias,
                                 scale=scale)

        def w_cols(pool, w_dram, kc, m0, msz, tag, c0=0, csz=None):
            """[kc*128, *] bf16 DRAM weight -> sbuf [P, csz, msz] col block."""
            csz = kc if csz is None else csz
            t = pool.tile([P, csz, msz], BF16, tag=tag,
                          bufs=(4 if csz <= 8 else 2), name=f"w_{tag}")
            src = w_dram.rearrange("(k p) m -> p k m", p=P)
            nc.sync.dma_start(out=t, in_=src[:, c0:c0 + csz, m0:m0 + msz])
            return t

        def mm_feature(wpool, w_dram, rhs_tiles, n_out, evict, wtag,
                       m_range=None, qs=(0, 1), rhs_half=False):
            """for m: psum[q] = W[:, mP:(m+1)P].T @ rhs[:, q-half];
            evict(m, q, ps[P,512])."""
            kc = len(rhs_tiles)
            KCH = 16
            for m in (range(n_out) if m_range is None else m_range):
                pss = {q: ps_tile(f"ps_{wtag}{m}_{q}") for q in qs}
                for c0 in range(0, kc, KCH):
                    cs = min(KCH, kc - c0)
                    wt = w_cols(wpool, w_dram, kc, m * P, P, wtag, c0, cs)
                    for d in range(cs):
                        for q in qs:
                            rhs = (rhs_tiles[c0 + d][:, 0:512] if rhs_half
                                   else rhs_tiles[c0 + d][:, QS[q]])
                            nc.tensor.matmul(
                                pss[q], wt[:, d, :], rhs,
                                start=(c0 + d == 0), stop=(c0 + d == kc - 1))
                for q in qs:
                    evict(m, q, pss[q])

        def layer_norm(ph, src, vec, s_col, b_col, name, out_pool=None,
                       xs_hook=None):
            """src: DRAM [DIM, TLOC] fp32 AP, or list of 8 sbuf fp32 tiles.
            Returns 8 bf16 [P, TLOC] normalized tiles (tag ln_xn)."""
            from_dram = callable(src)
            mean_ps = [ps1_tile(f"lnm_{name}{q}") for q in range(2)]
            sq_ps = [ps1_tile(f"lnsq_{name}{q}") for q in range(2)]
            xb_l = []
            for d in range(DC):
                if from_dram:
                    xsrc = ph.tile([P, TLOC], F32, tag="ln_xs", bufs=3,
                                   name=f"lnxs_{name}")
                    nc.sync.dma_start(out=xsrc, in_=src(d))
                else:
                    xsrc = src[d]
                if xs_hook is not None:
                    xs_hook(d, xsrc)
                xb = ph.tile([P, TLOC], BF16, tag="ln_xb", bufs=DC + 1,
                             name=f"lnxb_{name}")
                nc.vector.tensor_copy(xb, xsrc)
                xb_l.append(xb)
                sq = ph.tile([P, TLOC], BF16, tag="ln_sq", name=f"lnq_{name}")
                act(sq, xb, AF.Square)
                for q in range(2):
                    nc.tensor.matmul(mean_ps[q], ones128_bf, xb[:, QS[q]],
                                     start=(d == 0), stop=(d == DC - 1))
                    nc.tensor.matmul(sq_ps[q], ones128_bf, sq[:, QS[q]],
                                     start=(d == 0), stop=(d == DC - 1))
            nm = stats.tile([1, TLOC], F32, tag="st1", bufs=3, name=f"nm_{name}")
            msq = stats.tile([1, TLOC], F32, tag="st1", bufs=3,
                             name=f"msq_{name}")
            var = stats.tile([1, TLOC], F32, tag="st1", bufs=3,
                             name=f"var_{name}")
            rstd = stats.tile([1, 2, TLOC], F32, tag="st2", name=f"rstd_{name}")
            for q in range(2):
                act(nm[:, QS[q]], mean_ps[q], AF.Copy, scale=-1.0 / DIM)
                act(msq[:, QS[q]], sq_ps[q], AF.Copy, scale=1.0 / DIM)
            act(var, nm, AF.Square)
            nc.vector.tensor_sub(var, msq, var)
            lvar = stats.tile([1, TLOC], F32, tag="st1", bufs=3,
                              name=f"lv_{name}")
            act(lvar, var, AF.Ln, bias=eps_sb[:1])
            act(rstd[:, 0, :], lvar, AF.Exp, scale=-0.5)  # rsqrt(var+eps)
            nc.vector.tensor_mul(rstd[:, 1, :], nm, rstd[:, 0, :])
            bc = ph.tile([P, 2, TLOC], F32, tag="ln_bc", bufs=1,
                         name=f"bc_{name}")
            for j in range(2):
                for q in range(2):
                    bc_ps = ps_tile(f"lnbc_{name}{j}{q}")
                    nc.tensor.matmul(bc_ps, ones1_f32, rstd[:, j, QS[q]],
                                     start=True, stop=True)
                    act(bc[:, j, QS[q]], bc_ps)
            out_tiles = []
            for d in range(DC):
                xsrc = xb_l[d]
                t1 = ph.tile([P, TLOC], F32, tag="ln_t1", name=f"t1_{name}")
                nc.vector.tensor_mul(t1, xsrc, bc[:, 0, :])
                nc.vector.tensor_add(t1, t1, bc[:, 1, :])
                xn = (out_pool or ph).tile([P, TLOC], BF16, tag="ln_xn",
                                           bufs=DC + 1, name=f"xn_{name}")
                act(xn, t1, AF.Copy, bias=vec[:, d, b_col:b_col + 1],
                    scale=vec[:, d, s_col:s_col + 1])
                out_tiles.append(xn)
            return out_tiles

        # delayed = mean_S(x_dis): partial sums ride d0's LN1 stream, then
        # a pair AllReduce. dacc lives in singles; finalized in emit_d_qkv(0).
        delayed_sb = singles.tile([P, DC], F32)
        dacc = singles.tile([P, DC], F32, name="dacc")

        def delayed_hook(d, xsrc):
            r = stats.tile([P, 1], F32, tag="dred", bufs=3, name="dred")
            nc.vector.tensor_reduce(r, xsrc, axis=AX.X, op=ALU.add)
            nc.vector.tensor_scalar_mul(dacc[:, d:d + 1], r, 1.0 / S)

        def delayed_finalize():
            nc.sync.dma_start(out=cc_del_in[:, :], in_=dacc)
            nc.gpsimd.collective_compute(
                "AllReduce", ALU.add, replica_groups=PAIRS,
                ins=[cc_del_in[:, :]], outs=[cc_del_out[:, :]])
            nc.sync.dma_start(out=delayed_sb, in_=cc_del_out[:, :])

        def emit_e_ln(i, espan):
            vec = evec_sb[i]
            with tc.tile_pool(name=f"ph_e{i}l", bufs=1) as ph:
                return layer_norm(ph, lambda m, i=i: res_ap(e_src[i], m),
                                  vec, 0, 1, f"e{i}", out_pool=espan)

        def emit_e_mms(i, xn):
            vec = evec_sb[i]
            with tc.tile_pool(name=f"ph_e{i}m", bufs=1) as ph:
                h1 = []

                def ev_tanh(m, q, ps, vec=vec, ph=ph, h1=h1):
                    if q == 0:
                        h1.append(ph.tile([P, TLOC], BF16, tag="h1",
                                          bufs=DC + 1, name="h1"))
                    act(h1[m][:, QS[q]], ps, AF.Tanh, bias=vec[:, m, 2:3])
                mm_feature(ph, e_w1[i], xn, DC, ev_tanh, "w8a")

                xs_cur = {}

                def ev_e2(m, q, ps, vec=vec, ph=ph, i=i, xs_cur=xs_cur):
                    if q == 0:
                        xs = ph.tile([P, TLOC], F32, tag="xadd", bufs=3,
                                     name="xs")
                        nc.sync.dma_start(out=xs, in_=res_ap(e_src[i], m))
                        xs_cur[m] = xs
                    y = ph.tile([P, 512], F32, tag="y", bufs=4, name="y")
                    nc.vector.tensor_scalar(y, ps, common_sb[:, m, 0:1],
                                            vec[:, m, 3:4], ALU.mult, ALU.add)
                    xo = ph.tile([P, 512], F32, tag="xout", bufs=4, name="xo")
                    nc.vector.tensor_add(xo, xs_cur[m][:, QS[q]], y)
                    nc.sync.dma_start(
                        out=res_ap(e_dst[i], m)[:, QS[q]], in_=xo)
                mm_feature(ph, e_w2[i], h1, DC, ev_e2, "w8b")

        def emit_d_qkv(i, qpool):
            vec = dvec_sb[i]
            q_sb = [qpool.tile([P, TLOC], BF16, tag="q_sb", bufs=DC,
                               name=f"qsb{m}") for m in range(DC)]
            with tc.tile_pool(name=f"ph_d{i}a", bufs=1) as ph:
                xn1 = layer_norm(ph, lambda m, i=i: res_ap(d_src[i], m),
                                 vec, 0, 1, f"d{i}l1",
                                 xs_hook=(delayed_hook if i == 0 else None))
                if i == 0:
                    delayed_finalize()

                def ev_qk(m, q, ps, vec=vec, ph=ph, i=i):
                    if m < DC:
                        nc.vector.tensor_scalar(q_sb[m][:, QS[q]], ps,
                                                1.0 / 16.0, vec[:, m, 4:5],
                                                ALU.mult, ALU.add)
                        return
                    t = ph.tile([P, 512], BF16, tag="qk_ev", bufs=4,
                                name="qkev")
                    if True:
                        nc.vector.tensor_scalar_add(t, ps, vec[:, m - DC, 5:6])
                        nc.sync.dma_start(
                            out=k_loc[i][(m - DC) * P:(m - DC + 1) * P, QS[q]],
                            in_=t)
                mm_feature(ph, d_wqk[i], xn1, 2 * DC, ev_qk, "w8a",
                           m_range=range(DC, 2 * DC))

                nc.gpsimd.collective_compute(
                    "AllGather", ALU.bypass, replica_groups=PAIRS,
                    ins=[k_loc[i][:, :]], outs=[k_full[i][:, :, :]])

                # V token-major: lhsT = xn1 slices, rhs = Wv row-chunks
                wv_rows = []
                wv_src = d_wv[i].rearrange("(k p) m -> p k m", p=P)
                for d in range(DC):
                    t = ph.tile([P, DIM], BF16, tag="wv_row", bufs=DC,
                                name="wvr")
                    nc.sync.dma_start(out=t, in_=wv_src[:, d, :])
                    wv_rows.append(t)
                for m in range(DT):
                    pss = [ps_tile(f"ps_v{m}_{q}") for q in range(2)]
                    for d in range(DC):
                        for q in range(2):
                            nc.tensor.matmul(
                                pss[q], xn1[d][:, m * P:(m + 1) * P],
                                wv_rows[d][:, QS[q]],
                                start=(d == 0), stop=False)
                    for q in range(2):
                        nc.tensor.matmul(pss[q], ones1_bf, bv_sb[i][:, QS[q]],
                                         start=False, stop=True)
                        vt = ph.tile([P, 512], BF16, tag="qk_ev", bufs=4,
                                     name="vev")
                        nc.vector.tensor_copy(vt, pss[q])
                        nc.sync.dma_start(
                            out=v_loc[i][m * P:(m + 1) * P, QS[q]], in_=vt)

                nc.gpsimd.collective_compute(
                    "AllGather", ALU.bypass, replica_groups=PAIRS,
                    ins=[v_loc[i][:, :]], outs=[v_full[i][:, :, :]])

                mm_feature(ph, d_wqk[i], xn1, DC, ev_qk, "w8a",
                           m_range=range(DC))

            return q_sb

        def emit_d_att(i, q_sb, opool):
            o_sb = [None] * DC
            with tc.tile_pool(name=f"ph_d{i}b", bufs=1) as ph:
                v_res = []
                for kt in range(2 * DT):
                    r, m = kt // DT, kt % DT
                    t = ph.tile([P, DIM], BF16, tag="v_res", bufs=2 * DT,
                                name="vres")
                    nc.sync.dma_start(
                        out=t, in_=v_full[i][r, m * P:(m + 1) * P, :])
                    v_res.append(t)
                for h in range(NHEAD):
                    qh = [q_sb[2 * h + s] for s in range(2)]
                    av_ps = [[ps_tile(f"av{h}_{s2}_{q}") for q in range(2)]
                             for s2 in range(2)]
                    den_ps = [ps1_tile(f"den{h}_{q}") for q in range(2)]
                    for kt_g in range(4):
                        r, cb = kt_g // 2, (kt_g % 2) * 512
                        kTs = []
                        for s in range(2):
                            t = ph.tile([P, 512], BF16, tag="kT", bufs=4,
                                        name="kT")
                            fr = h * HD + s * P
                            nc.gpsimd.dma_start(
                                out=t,
                                in_=k_full[i][r, fr:fr + P, cb:cb + 512])
                            kTs.append(t)
                        for j in range(4):
                            kt = kt_g * 4 + j
                            first, last = kt == 0, kt == 2 * DT - 1
                            for q in range(2):
                                sc = ps_tile(f"sc{h}_{kt}_{q}")
                                for s in range(2):
                                    nc.tensor.matmul(
                                        sc, kTs[s][:, j * P:(j + 1) * P],
                                        qh[s][:, QS[q]],
                                        start=(s == 0), stop=(s == 1))
                                et = ph.tile([P, 512], BF16, tag="expT",
                                             bufs=6, name="et")
                                act(et, sc, AF.Exp)
                                nc.tensor.matmul(den_ps[q], ones128_bf, et,
                                                 start=first, stop=last)
                                for s2 in range(2):
                                    fr = h * HD + s2 * P
                                    nc.tensor.matmul(
                                        av_ps[s2][q],
                                        v_res[kt][:, fr:fr + P], et,
                                        start=first, stop=last)
                    lden = stats.tile([1, TLOC], F32, tag="st1", bufs=3,
                                      name="lden")
                    rden = stats.tile([1, TLOC], BF16, tag="rden", bufs=2,
                                      name="rden")
                    rb = ph.tile([P, TLOC], F32, tag="rb", bufs=2, name="rb")
                    for q in range(2):
                        act(lden[:, QS[q]], den_ps[q], AF.Ln)
                        act(rden[:, QS[q]], lden[:, QS[q]], AF.Exp,
                            scale=-1.0)  # 1/den
                        rb_ps = ps_tile(f"rb{h}{q}")
                        nc.tensor.matmul(rb_ps, ones1_bf, rden[:, QS[q]],
                                         start=True, stop=True)
                        act(rb[:, QS[q]], rb_ps)
                    for s2 in range(2):
                        ot = opool.tile([P, TLOC], BF16, tag="ot", bufs=DC,
                                        name="ot")
                        for q in range(2):
                            nc.vector.tensor_mul(ot[:, QS[q]], av_ps[s2][q],
                                                 rb[:, QS[q]])
                        o_sb[2 * h + s2] = ot
            return o_sb

        def emit_d_tail(i, o_sb, blk, otpool):
            vec = dvec_sb[i]
            bias_d = singles.tile([P, DC], F32, tag=f"biasd{i}",
                                  name=f"biasd{i}")
            nc.vector.tensor_scalar_mul(bias_d, delayed_sb, 0.3)
            nc.vector.tensor_add(bias_d, bias_d, vec[:, :, 6])
            nc.vector.tensor_mul(bias_d, bias_d, common_sb[:, :, 1])
            if True:
                hmid = []
                xn2 = None
                with tc.tile_pool(name=f"ph_d{i}t1", bufs=1) as ph:
                    xs_cur = {}

                    def ev_out(m, q, ps, vec=vec, ph=ph, i=i):
                        if q == 0:
                            xs = ph.tile([P, TLOC], F32, tag="xadd", bufs=3,
                                         name="xs")
                            nc.sync.dma_start(
                                out=xs, in_=res_ap(d_src[i], m))
                            xs_cur[m] = xs
                            hmid.append(blk.tile([P, TLOC], F32, tag="hmid",
                                                 bufs=DC, name="hm"))
                        y = ph.tile([P, 512], F32, tag="y", bufs=4, name="y")
                        nc.vector.tensor_scalar(y, ps, common_sb[:, m, 1:2],
                                                bias_d[:, m:m + 1], ALU.mult,
                                                ALU.add)
                        nc.vector.tensor_add(hmid[m][:, QS[q]],
                                             xs_cur[m][:, QS[q]], y)
                    mm_feature(ph, d_wout[i], o_sb, DC, ev_out, "w8b")
                otpool.release()

                with tc.tile_pool(name=f"ph_d{i}t2", bufs=1) as ph:
                    xn2 = layer_norm(ph, hmid, vec, 2, 3, f"d{i}l2",
                                     out_pool=blk)

                with tc.tile_pool(name=f"ph_d{i}t3", bufs=1) as ph:
                    for qh in range(2):
                        g_half = ph.tile([P, FC, 512], BF16, tag="g_half",
                                         bufs=1, name="g")

                        def ev_g(m, q, ps, i=i, g_half=g_half):
                            act(g_half[:, m, :], ps, AF.Gelu,
                                bias=ff1b_sb[i][:, m:m + 1])
                        mm_feature(ph, d_ff1[i], xn2, FC, ev_g, "w8a",
                                   qs=(qh,))

                        g_tiles = [g_half[:, m, :] for m in range(FC)]

                        def ev_f2(m, q, ps, vec=vec, ph=ph, i=i):
                            y = ph.tile([P, 512], F32, tag="y", bufs=3,
                                        name="y")
                            nc.vector.tensor_scalar(y, ps, 0.5, vec[:, m, 7:8],
                                                    ALU.mult, ALU.add)
                            xo = ph.tile([P, 512], F32, tag="xout", bufs=3,
                                         name="xo")
                            nc.vector.tensor_add(xo, hmid[m][:, QS[q]], y)
                            nc.sync.dma_start(
                                out=res_ap(d_dst[i], m)[:, QS[q]], in_=xo)
                        mm_feature(ph, d_ff2[i], g_tiles, DC, ev_f2, "w32",
                                   qs=(qh,), rhs_half=True)

        # e-LN hoisted ahead of d-qkv (runs concurrently); e-mms fill the
        # gather windows; q stays SBUF-resident through attention.
        for i in range(kk):
            xn_e = emit_e_ln(i, espan)
            q_sb = emit_d_qkv(i, qpool)   # ends with the AllGather issues
            emit_e_mms(i, xn_e)
            blk = tc.alloc_tile_pool(name=f"ph_d{i}blk", bufs=1)
            otpool = tc.alloc_tile_pool(name=f"ot{i}", bufs=1)
            o_sb = emit_d_att(i, q_sb, otpool)
            emit_d_tail(i, o_sb, blk, otpool)
            blk.release()

    nc.compile()
    return nc


# ---------------------------------------------------------------------------
# host side
# ---------------------------------------------------------------------------
_prog_cache = {}


def _pack_pvec(v):
    """[1024] -> [128, 8]: out[p, i] = v[i*128 + p]."""
    return np.ascontiguousarray(np.asarray(v, np.float32).reshape(-1, P).T)


def _bf(x):
    return np.ascontiguousarray(x).astype(ml_dtypes.bfloat16)


def kernel(**inputs):
    inp = {k: np.asarray(v) for k, v in inputs.items()}
    kk = min(int(inp["max_active_blocks"]), NBLK)
    emb = inp["embodied_input"].astype(np.float32)
    dis = inp["disembodied_input"].astype(np.float32)
    tf = inp["torsion_field"].astype(np.float32)

    if kk == 0:
        return emb.copy(), dis.copy()

    def sigmoid(x):
        return 1.0 / (1.0 + np.exp(-x))

    esc_s = sigmoid(emb[0].mean(axis=0) @ inp["esel_w"].T + inp["esel_b"])
    etop = np.argsort(-esc_s, kind="stable")[:kk]
    dsc_s = sigmoid(dis[0].mean(axis=0) @ inp["dsel_w"].T + inp["dsel_b"])
    dtop = np.argsort(-dsc_s, kind="stable")[:kk]

    if kk not in _prog_cache:
        _prog_cache[kk] = build_program(kk)
    nc = _prog_cache[kk]

    wmap = {}
    for i, idx in enumerate(etop):
        wmap[f"e{i}_w1"] = _bf(inp["e_w1"][idx].T)
        wmap[f"e{i}_w2"] = _bf(inp["e_w2"][idx].T)
    for i, idx in enumerate(dtop):
        qkv_w = inp["d_qkv_w"][idx]  # [3072, 1024]
        qkv_b = inp["d_qkv_b"][idx]
        wmap[f"d{i}_wqk"] = _bf(qkv_w[:2 * DIM].T)
        wmap[f"d{i}_wv"] = _bf(qkv_w[2 * DIM:].T)
        wmap[f"d{i}_wout"] = _bf(inp["d_out_w"][idx].T)
        wmap[f"d{i}_ff1"] = _bf(inp["d_ff1_w"][idx].T)
        wmap[f"d{i}_ff2"] = _bf(inp["d_ff2_w"][idx].T)
        wmap[f"d{i}_vec"] = np.stack([
            _pack_pvec(inp["d_ln1_s"][idx]),
            _pack_pvec(inp["d_ln1_b"][idx]),
            _pack_pvec(inp["d_ln2_s"][idx]),
            _pack_pvec(inp["d_ln2_b"][idx]),
            _pack_pvec(qkv_b[:DIM] / 16.0),
            _pack_pvec(qkv_b[DIM:2 * DIM]),
            _pack_pvec(inp["d_out_b"][idx]),
            _pack_pvec(0.5 * inp["d_ff2_b"][idx]),
        ], axis=-1)
        wmap[f"d{i}_ff1b"] = _pack_pvec(inp["d_ff1_b"][idx])
        wmap[f"d{i}_bv"] = _bf(qkv_b[2 * DIM:].reshape(1, DIM))

    in_maps = []
    for c in range(NCORES):
        b, half = c // 2, c % 2
        tsl = slice(half * TLOC, (half + 1) * TLOC)
        m = dict(wmap)
        m["x_emb"] = np.ascontiguousarray(emb[b, tsl].T)
        m["x_dis"] = np.ascontiguousarray(dis[b, tsl].T)
        esc_v = 0.3 * (1.0 + 0.1 * tf[b])
        s05_v = 0.5 * (1.0 + 0.05 * tf[b])
        m["common"] = np.stack([_pack_pvec(esc_v), _pack_pvec(s05_v)], axis=-1)
        for i, idx in enumerate(etop):
            m[f"e{i}_vec"] = np.stack([
                _pack_pvec(inp["e_ln_s"][idx]),
                _pack_pvec(inp["e_ln_b"][idx]),
                _pack_pvec(inp["e_b1"][idx]),
                _pack_pvec(esc_v * inp["e_b2"][idx]),
            ], axis=-1)
        in_maps.append(m)

    from concourse.bass_utils import run_bass_kernel_spmd
    res = run_bass_kernel_spmd(nc, in_maps, list(range(NCORES)))

    h = np.empty((B, S, DIM), np.float32)
    dh = np.empty((B, S, DIM), np.float32)
    for c in range(NCORES):
        b, half = c // 2, c % 2
        tsl = slice(half * TLOC, (half + 1) * TLOC)
        h[b, tsl] = res.results[c]["out_e"].T
        dh[b, tsl] = res.results[c]["out_d"].T
    return h, dh



# revision 46
# speedup vs baseline: 1.0202x; 1.0202x over previous
"""Trainium2 Bass kernel for nn_DevelopmentalLayerV51 (moe_routing).

kernel(**inputs) takes the FULL unsharded inputs (as reference.setup_inputs)
and returns the full (h, dh) tuple of np.float32 arrays.

Sharding: data-parallel over the B*S=8192 tokens across 8 cores (core c owns
batch c//2, sequence half c%2 -> 1024 tokens). Top-k routing uses only
scores[0], so it is decided on host; only the selected blocks' weights are
shipped (replicated, bf16). On device all activations are feature-major
[D, tokens] so every matmul is transpose-free (contraction dim in
partitions for both operands); LayerNorm/softmax partition-reductions use
ones-matmuls on the PE. Attention K/V are AllGathered within the core pair
sharing a batch; delayed = mean_S(disembodied) via a tiny pair AllReduce.
Residual streams stay fp32 and round-trip through DRAM between phases so
SBUF only ever holds one phase's working set.
"""
import sys

sys.path.insert(0, "/opt/trn_rl_repo")

import contextlib

import numpy as np
import ml_dtypes

import concourse.bass as bass
import concourse.tile as tile
from concourse import bacc, mybir

DIM = 1024
NBLK = 8
NHEAD = 4
HD = DIM // NHEAD          # 256
B, S = 4, 2048
NCORES = 8
TLOC = (B * S) // NCORES   # 1024 tokens per core
P = 128
DC = DIM // P              # 8 feature tiles
DT = TLOC // P             # 8 token tiles
FF = 4 * DIM               # 4096
FC = FF // P               # 32
F32 = mybir.dt.float32
BF16 = mybir.dt.bfloat16
AF = mybir.ActivationFunctionType
AX = mybir.AxisListType
ALU = mybir.AluOpType

PAIRS = [[0, 1], [2, 3], [4, 5], [6, 7]]
QS = [slice(0, 512), slice(512, 1024)]


def build_program(kk):
    nc = bacc.Bacc("TRN2", target_bir_lowering=False, debug=False,
                   num_devices=NCORES)
    dp = nc.declare_dram_parameter

    x_emb = dp("x_emb", [DIM, TLOC], F32, isOutput=False)
    x_dis = dp("x_dis", [DIM, TLOC], F32, isOutput=False)
    common = dp("common", [P, DC, 2], F32, isOutput=False)  # esc, s05
    e_w1 = [dp(f"e{i}_w1", [DIM, DIM], BF16, isOutput=False) for i in range(kk)]
    e_w2 = [dp(f"e{i}_w2", [DIM, DIM], BF16, isOutput=False) for i in range(kk)]
    # cols: ln_s, ln_b, b1, esc*b2
    e_vec = [dp(f"e{i}_vec", [P, DC, 4], F32, isOutput=False) for i in range(kk)]
    d_wqk = [dp(f"d{i}_wqk", [DIM, 2 * DIM], BF16, isOutput=False) for i in range(kk)]
    d_wv = [dp(f"d{i}_wv", [DIM, DIM], BF16, isOutput=False) for i in range(kk)]
    d_wout = [dp(f"d{i}_wout", [DIM, DIM], BF16, isOutput=False) for i in range(kk)]
    d_ff1 = [dp(f"d{i}_ff1", [DIM, FF], BF16, isOutput=False) for i in range(kk)]
    d_ff2 = [dp(f"d{i}_ff2", [FF, DIM], BF16, isOutput=False) for i in range(kk)]
    # cols: ln1_s, ln1_b, ln2_s, ln2_b, bq/16, bk, b_out, 0.5*b_ff2
    d_vec = [dp(f"d{i}_vec", [P, DC, 8], F32, isOutput=False) for i in range(kk)]
    d_ff1b = [dp(f"d{i}_ff1b", [P, FC], F32, isOutput=False) for i in range(kk)]
    d_bv = [dp(f"d{i}_bv", [1, DIM], BF16, isOutput=False) for i in range(kk)]
    out_e = dp("out_e", [DIM, TLOC], F32, isOutput=True)
    out_d = dp("out_d", [DIM, TLOC], F32, isOutput=True)

    dt_ = nc.dram_tensor
    cc_del_in = dt_("cc_del_in", [P, DC], F32)
    cc_del_out = dt_("cc_del_out", [P, DC], F32)
    e_mid = [[dt_(f"e_mid{i}_{m}", [P, TLOC], F32) for m in range(DC)]
             for i in range(max(kk - 1, 0))]
    d_mid = [[dt_(f"d_mid{i}_{m}", [P, TLOC], F32) for m in range(DC)]
             for i in range(max(kk - 1, 0))]
    k_loc = [dt_(f"k_loc{i}", [DIM, TLOC], BF16) for i in range(kk)]
    v_loc = [dt_(f"v_loc{i}", [DIM, TLOC], BF16) for i in range(kk)]
    k_full = [dt_(f"k_full{i}", [2, DIM, TLOC], BF16) for i in range(kk)]
    v_full = [dt_(f"v_full{i}", [2, DIM, TLOC], BF16) for i in range(kk)]

    def res_ap(t, m):
        return t[m][:, :] if isinstance(t, list) else t[m * P:(m + 1) * P, :]

    e_src = [x_emb] + e_mid
    e_dst = e_mid + [out_e]
    d_src = [x_dis] + d_mid
    d_dst = d_mid + [out_d]

    with tile.TileContext(nc, pool_alloc_mode="queue") as tc, \
         contextlib.ExitStack() as octx:
        singles = octx.enter_context(tc.tile_pool(name="singles", bufs=1))
        stats = octx.enter_context(tc.tile_pool(name="stats", bufs=1))
        qpool = octx.enter_context(tc.tile_pool(name="qp", bufs=1))
        espan = octx.enter_context(tc.tile_pool(name="esp", bufs=1))
        h1pool = octx.enter_context(tc.tile_pool(name="h1p", bufs=1))
        psum = octx.enter_context(tc.tile_pool(name="psum", bufs=8,
                                               space="PSUM"))
        e_xn = {}
        e_h1 = [[] for _ in range(kk)]

        def ps_tile(name):
            return psum.tile([P, 512], F32, tag="ps", name=name)

        def ps1_tile(name):
            return psum.tile([1, 512], F32, tag="ps", name=name)

        ones128_bf = singles.tile([P, 1], BF16)
        nc.vector.memset(ones128_bf, 1.0)
        ones1_f32 = singles.tile([1, P], F32)
        nc.vector.memset(ones1_f32, 1.0)
        ones1_bf = singles.tile([1, P], BF16)
        nc.vector.memset(ones1_bf, 1.0)

        eps_sb = singles.tile([P, 1], F32)
        nc.vector.memset(eps_sb, 1e-5)
        common_sb = singles.tile([P, DC, 2], F32)
        nc.sync.dma_start(out=common_sb, in_=common[:, :, :])
        evec_sb = []
        dvec_sb = []
        ff1b_sb = []
        bv_sb = []
        for i in range(kk):
            t = singles.tile([P, DC, 4], F32, name=f"evec{i}", tag=f"evec{i}")
            nc.sync.dma_start(out=t, in_=e_vec[i][:, :, :])
            evec_sb.append(t)
            t = singles.tile([P, DC, 8], F32, name=f"dvec{i}", tag=f"dvec{i}")
            nc.sync.dma_start(out=t, in_=d_vec[i][:, :, :])
            dvec_sb.append(t)
            t = singles.tile([P, FC], F32, name=f"ff1b{i}", tag=f"ff1b{i}")
            nc.sync.dma_start(out=t, in_=d_ff1b[i][:, :])
            ff1b_sb.append(t)
            t = singles.tile([1, DIM], BF16, name=f"bv{i}", tag=f"bv{i}")
            nc.sync.dma_start(out=t, in_=d_bv[i][:, :])
            bv_sb.append(t)

        def act(out, in_, func=AF.Copy, bias=0.0, scale=1.0):
            if func == AF.Copy and not isinstance(bias, float):
                func = AF.Identity  # Copy rejects AP bias; Identity is affine
            nc.scalar.activation(out=out, in_=in_, func=func, bias=bias,
                                 scale=scale)

        def w_cols(pool, w_dram, kc, m0, msz, tag, c0=0, csz=None):
            """[kc*128, *] bf16 DRAM weight -> sbuf [P, csz, msz] col block."""
            csz = kc if csz is None else csz
            t = pool.tile([P, csz, msz], BF16, tag=tag,
                          bufs=(4 if csz <= 8 else 2), name=f"w_{tag}")
            src = w_dram.rearrange("(k p) m -> p k m", p=P)
            nc.sync.dma_start(out=t, in_=src[:, c0:c0 + csz, m0:m0 + msz])
            return t

        def mm_feature(wpool, w_dram, rhs_tiles, n_out, evict, wtag,
                       m_range=None, qs=(0, 1), rhs_half=False):
            """for m: psum[q] = W[:, mP:(m+1)P].T @ rhs[:, q-half];
            evict(m, q, ps[P,512])."""
            kc = len(rhs_tiles)
            KCH = 16
            for m in (range(n_out) if m_range is None else m_range):
                pss = {q: ps_tile(f"ps_{wtag}{m}_{q}") for q in qs}
                for c0 in range(0, kc, KCH):
                    cs = min(KCH, kc - c0)
                    wt = w_cols(wpool, w_dram, kc, m * P, P, wtag, c0, cs)
                    for d in range(cs):
                        for q in qs:
                            rhs = (rhs_tiles[c0 + d][:, 0:512] if rhs_half
                                   else rhs_tiles[c0 + d][:, QS[q]])
                            nc.tensor.matmul(
                                pss[q], wt[:, d, :], rhs,
                                start=(c0 + d == 0), stop=(c0 + d == kc - 1))
                for q in qs:
                    evict(m, q, pss[q])

        def layer_norm(ph, src, vec, s_col, b_col, name, out_pool=None,
                       xs_hook=None, filler1=None, filler2=None):
            """src: DRAM [DIM, TLOC] fp32 AP, or list of 8 sbuf fp32 tiles.
            Returns 8 bf16 [P, TLOC] normalized tiles (tag ln_xn).
            filler1/filler2: callables emitting PE work to cover the
            var/rstd chain and the per-tile normalize wave."""
            from_dram = callable(src)
            mean_ps = [ps1_tile(f"lnm_{name}{q}") for q in range(2)]
            sq_ps = [ps1_tile(f"lnsq_{name}{q}") for q in range(2)]
            xb_l = []
            for d in range(DC):
                if from_dram:
                    xsrc = ph.tile([P, TLOC], F32, tag="ln_xs", bufs=3,
                                   name=f"lnxs_{name}")
                    nc.sync.dma_start(out=xsrc, in_=src(d))
                else:
                    xsrc = src[d]
                if xs_hook is not None:
                    xs_hook(d, xsrc)
                xb = ph.tile([P, TLOC], BF16, tag="ln_xb", bufs=DC + 1,
                             name=f"lnxb_{name}")
                nc.vector.tensor_copy(xb, xsrc)
                xb_l.append(xb)
                sq = ph.tile([P, TLOC], BF16, tag="ln_sq", name=f"lnq_{name}")
                act(sq, xb, AF.Square)
                for q in range(2):
                    nc.tensor.matmul(mean_ps[q], ones128_bf, xb[:, QS[q]],
                                     start=(d == 0), stop=(d == DC - 1))
                    nc.tensor.matmul(sq_ps[q], ones128_bf, sq[:, QS[q]],
                                     start=(d == 0), stop=(d == DC - 1))
            nm = stats.tile([1, TLOC], F32, tag="st1", bufs=3, name=f"nm_{name}")
            msq = stats.tile([1, TLOC], F32, tag="st1", bufs=3,
                             name=f"msq_{name}")
            var = stats.tile([1, TLOC], F32, tag="st1", bufs=3,
                             name=f"var_{name}")
            std = stats.tile([1, TLOC], F32, tag="st1", bufs=3,
                             name=f"std_{name}")
            rstd = stats.tile([1, 2, TLOC], F32, tag="st2", name=f"rstd_{name}")
            for q in range(2):
                act(nm[:, QS[q]], mean_ps[q], AF.Copy, scale=-1.0 / DIM)
                act(msq[:, QS[q]], sq_ps[q], AF.Copy, scale=1.0 / DIM)
            act(var, nm, AF.Square)
            nc.vector.tensor_sub(var, msq, var)
            act(std, var, AF.Sqrt, bias=eps_sb[:1])
            nc.vector.reciprocal(rstd[:, 0, :], std)
            nc.vector.tensor_mul(rstd[:, 1, :], nm, rstd[:, 0, :])
            if filler1 is not None:
                filler1()
            bc = ph.tile([P, 2, TLOC], F32, tag="ln_bc", bufs=1,
                         name=f"bc_{name}")
            for j in range(2):
                for q in range(2):
                    bc_ps = ps_tile(f"lnbc_{name}{j}{q}")
                    nc.tensor.matmul(bc_ps, ones1_f32, rstd[:, j, QS[q]],
                                     start=True, stop=True)
                    act(bc[:, j, QS[q]], bc_ps)
            if filler2 is not None:
                filler2()
            out_tiles = []
            for d in range(DC):
                xsrc = xb_l[d]
                t1 = ph.tile([P, TLOC], F32, tag="ln_t1", name=f"t1_{name}")
                nc.vector.tensor_mul(t1, xsrc, bc[:, 0, :])
                nc.vector.tensor_add(t1, t1, bc[:, 1, :])
                xn = (out_pool or ph).tile([P, TLOC], BF16, tag="ln_xn",
                                           bufs=DC + 1, name=f"xn_{name}")
                act(xn, t1, AF.Copy, bias=vec[:, d, b_col:b_col + 1],
                    scale=vec[:, d, s_col:s_col + 1])
                out_tiles.append(xn)
            return out_tiles

        # delayed = mean_S(x_dis): partial sums ride d0's LN1 stream, then
        # a pair AllReduce. dacc lives in singles; finalized in emit_d_qkv(0).
        delayed_sb = singles.tile([P, DC], F32)
        dacc = singles.tile([P, DC], F32, name="dacc")

        def delayed_hook(d, xsrc):
            r = stats.tile([P, 1], F32, tag="dred", bufs=3, name="dred")
            nc.vector.tensor_reduce(r, xsrc, axis=AX.X, op=ALU.add)
            nc.vector.tensor_scalar_mul(dacc[:, d:d + 1], r, 1.0 / S)

        def delayed_finalize():
            nc.sync.dma_start(out=cc_del_in[:, :], in_=dacc)
            nc.gpsimd.collective_compute(
                "AllReduce", ALU.add, replica_groups=PAIRS,
                ins=[cc_del_in[:, :]], outs=[cc_del_out[:, :]])
            nc.sync.dma_start(out=delayed_sb, in_=cc_del_out[:, :])

        def emit_e_ln(i, espan):
            vec = evec_sb[i]
            with tc.tile_pool(name=f"ph_e{i}l", bufs=1) as ph:
                return layer_norm(ph, lambda m, i=i: res_ap(e_src[i], m),
                                  vec, 0, 1, f"e{i}", out_pool=espan)

        def emit_e_w1(i, xn, wpool, m_range):
            """tanh(W1 @ xn) -> h1 tiles (persistent h1pool)."""
            vec = evec_sb[i]
            h1 = e_h1[i]

            def ev_tanh(m, q, ps, vec=vec):
                if q == 0:
                    while len(h1) <= m:
                        h1.append(h1pool.tile([P, TLOC], BF16, tag="h1",
                                              bufs=DC + 1, name="h1"))
                act(h1[m][:, QS[q]], ps, AF.Tanh, bias=vec[:, m, 2:3])
            mm_feature(wpool, e_w1[i], xn, DC, ev_tanh, "w8a",
                       m_range=m_range)

        def emit_e_w2(i, wpool, m_range):
            vec = evec_sb[i]
            xs_cur = {}

            def ev_e2(m, q, ps, vec=vec, i=i, xs_cur=xs_cur):
                if q == 0:
                    xs = wpool.tile([P, TLOC], F32, tag="xadd_e", bufs=3,
                                    name="xs")
                    nc.sync.dma_start(out=xs, in_=res_ap(e_src[i], m))
                    xs_cur[m] = xs
                y = wpool.tile([P, 512], F32, tag="y_e", bufs=3, name="y")
                nc.vector.tensor_scalar(y, ps, common_sb[:, m, 0:1],
                                        vec[:, m, 3:4], ALU.mult, ALU.add)
                xo = wpool.tile([P, 512], F32, tag="xout_e", bufs=3, name="xo")
                nc.vector.tensor_add(xo, xs_cur[m][:, QS[q]], y)
                nc.sync.dma_start(
                    out=res_ap(e_dst[i], m)[:, QS[q]], in_=xo)
            mm_feature(wpool, e_w2[i], e_h1[i], DC, ev_e2, "w8b",
                       m_range=m_range)

        def emit_d_qkv(i, qpool):
            vec = dvec_sb[i]
            q_sb = [qpool.tile([P, TLOC], BF16, tag="q_sb", bufs=DC,
                               name=f"qsb{m}") for m in range(DC)]
            with tc.tile_pool(name=f"ph_d{i}a", bufs=1) as ph:
                emit_e_w1(i, e_xn[i], ph, range(0, 5))
                xn1 = layer_norm(
                    ph, lambda m, i=i: res_ap(d_src[i], m),
                    vec, 0, 1, f"d{i}l1",
                    xs_hook=(delayed_hook if i == 0 else None),
                    filler1=lambda: emit_e_w1(i, e_xn[i], ph, range(5, DC)))
                if i == 0:
                    delayed_finalize()

                def ev_qk(m, q, ps, vec=vec, ph=ph, i=i):
                    if m < DC:
                        nc.vector.tensor_scalar(q_sb[m][:, QS[q]], ps,
                                                1.0 / 16.0, vec[:, m, 4:5],
                                                ALU.mult, ALU.add)
                        return
                    t = ph.tile([P, 512], BF16, tag="qk_ev", bufs=4,
                                name="qkev")
                    if True:
                        nc.vector.tensor_scalar_add(t, ps, vec[:, m - DC, 5:6])
                        nc.sync.dma_start(
                            out=k_loc[i][(m - DC) * P:(m - DC + 1) * P, QS[q]],
                            in_=t)
                mm_feature(ph, d_wqk[i], xn1, 2 * DC, ev_qk, "w8a",
                           m_range=range(DC, 2 * DC))

                nc.gpsimd.collective_compute(
                    "AllGather", ALU.bypass, replica_groups=PAIRS,
                    ins=[k_loc[i][:, :]], outs=[k_full[i][:, :, :]])

                # V token-major: lhsT = xn1 slices, rhs = Wv row-chunks
                wv_rows = []
                wv_src = d_wv[i].rearrange("(k p) m -> p k m", p=P)
                for d in range(DC):
                    t = ph.tile([P, DIM], BF16, tag="wv_row", bufs=DC,
                                name="wvr")
                    nc.sync.dma_start(out=t, in_=wv_src[:, d, :])
                    wv_rows.append(t)
                for m in range(DT):
                    pss = [ps_tile(f"ps_v{m}_{q}") for q in range(2)]
                    for d in range(DC):
                        for q in range(2):
                            nc.tensor.matmul(
                                pss[q], xn1[d][:, m * P:(m + 1) * P],
                                wv_rows[d][:, QS[q]],
                                start=(d == 0), stop=False)
                    for q in range(2):
                        nc.tensor.matmul(pss[q], ones1_bf, bv_sb[i][:, QS[q]],
                                         start=False, stop=True)
                        vt = ph.tile([P, 512], BF16, tag="qk_ev", bufs=4,
                                     name="vev")
                        nc.vector.tensor_copy(vt, pss[q])
                        nc.sync.dma_start(
                            out=v_loc[i][m * P:(m + 1) * P, QS[q]], in_=vt)

                nc.gpsimd.collective_compute(
                    "AllGather", ALU.bypass, replica_groups=PAIRS,
                    ins=[v_loc[i][:, :]], outs=[v_full[i][:, :, :]])

                mm_feature(ph, d_wqk[i], xn1, DC, ev_qk, "w8a",
                           m_range=range(DC))

            return q_sb

        def emit_d_att(i, q_sb, opool):
            o_sb = [None] * DC
            with tc.tile_pool(name=f"ph_d{i}b", bufs=1) as ph:
                pend = None

                def head_tail(h, av_sb, rden):
                    rb = ph.tile([P, TLOC], BF16, tag="rb", bufs=2, name="rb")
                    for q in range(2):
                        rb_ps = ps_tile(f"rb{h}{q}")
                        nc.tensor.matmul(rb_ps, ones1_bf, rden[:, QS[q]],
                                         start=True, stop=True)
                        act(rb[:, QS[q]], rb_ps)
                    for s2 in range(2):
                        ot = opool.tile([P, TLOC], BF16, tag="ot", bufs=DC,
                                        name="ot")
                        nc.vector.tensor_mul(ot, av_sb[s2], rb)
                        o_sb[2 * h + s2] = ot

                for h in range(NHEAD):
                    v_res = []
                    for kt in range(2 * DT):
                        r, m = kt // DT, kt % DT
                        t = ph.tile([P, HD], BF16, tag="v_res",
                                    bufs=2 * DT + 4, name="vres")
                        nc.sync.dma_start(
                            out=t, in_=v_full[i][r, m * P:(m + 1) * P,
                                                 h * HD:(h + 1) * HD])
                        v_res.append(t)
                    qh = [q_sb[2 * h + s] for s in range(2)]
                    av_ps = [[ps_tile(f"av{h}_{s2}_{q}") for q in range(2)]
                             for s2 in range(2)]
                    den_ps = [ps1_tile(f"den{h}_{q}") for q in range(2)]
                    for kt_g in range(4):
                        r, cb = kt_g // 2, (kt_g % 2) * 512
                        kTs = []
                        for s in range(2):
                            t = ph.tile([P, 512], BF16, tag="kT", bufs=4,
                                        name="kT")
                            fr = h * HD + s * P
                            nc.gpsimd.dma_start(
                                out=t,
                                in_=k_full[i][r, fr:fr + P, cb:cb + 512])
                            kTs.append(t)
                        for j in range(4):
                            kt = kt_g * 4 + j
                            first, last = kt == 0, kt == 2 * DT - 1
                            for q in range(2):
                                sc = ps_tile(f"sc{h}_{kt}_{q}")
                                for s in range(2):
                                    nc.tensor.matmul(
                                        sc, kTs[s][:, j * P:(j + 1) * P],
                                        qh[s][:, QS[q]],
                                        start=(s == 0), stop=(s == 1))
                                et = ph.tile([P, 512], BF16, tag="expT",
                                             bufs=6, name="et")
                                act(et, sc, AF.Exp)
                                nc.tensor.matmul(den_ps[q], ones128_bf, et,
                                                 start=first, stop=last)
                                for s2 in range(2):
                                    nc.tensor.matmul(
                                        av_ps[s2][q],
                                        v_res[kt][:, s2 * P:(s2 + 1) * P], et,
                                        start=first, stop=last)
                    # free PSUM fast: den -> sbuf (Scalar copy), av -> sbuf
                    # (DVE copy, unnormalized); the slow DVE reciprocal and
                    # the rb/ot tail run during the NEXT head's matmuls.
                    den_sb = stats.tile([1, TLOC], F32, tag="densb", bufs=2,
                                        name="densb")
                    rden = stats.tile([1, TLOC], BF16, tag="rden", bufs=2,
                                      name="rden")
                    for q in range(2):
                        act(den_sb[:, QS[q]], den_ps[q])
                    av_sb = [ph.tile([P, TLOC], BF16, tag="av_sb", bufs=5,
                                     name="avsb") for _ in range(2)]
                    for s2 in range(2):
                        for q in range(2):
                            nc.vector.tensor_copy(av_sb[s2][:, QS[q]],
                                                  av_ps[s2][q])
                    with nc.allow_low_precision(reason="1/den bf16 bcast"):
                        for q in range(2):
                            nc.vector.reciprocal(rden[:, QS[q]],
                                                 den_sb[:, QS[q]])
                    if pend is not None:
                        head_tail(*pend)
                    pend = (h, av_sb, rden)
                head_tail(*pend)
            return o_sb

        def emit_d_tail(i, o_sb, blk, otpool):
            vec = dvec_sb[i]
            bias_d = singles.tile([P, DC], F32, tag=f"biasd{i}",
                                  name=f"biasd{i}")
            nc.vector.tensor_scalar_mul(bias_d, delayed_sb, 0.3)
            nc.vector.tensor_add(bias_d, bias_d, vec[:, :, 6])
            nc.vector.tensor_mul(bias_d, bias_d, common_sb[:, :, 1])
            if True:
                hmid = []
                xn2 = None
                with tc.tile_pool(name=f"ph_d{i}t1", bufs=1) as ph:
                    xs_cur = {}

                    def ev_out(m, q, ps, vec=vec, ph=ph, i=i):
                        if q == 0:
                            xs = ph.tile([P, TLOC], F32, tag="xadd", bufs=3,
                                         name="xs")
                            nc.sync.dma_start(
                                out=xs, in_=res_ap(d_src[i], m))
                            xs_cur[m] = xs
                            hmid.append(blk.tile([P, TLOC], F32, tag="hmid",
                                                 bufs=DC, name="hm"))
                        y = ph.tile([P, 512], F32, tag="y", bufs=4, name="y")
                        nc.vector.tensor_scalar(y, ps, common_sb[:, m, 1:2],
                                                bias_d[:, m:m + 1], ALU.mult,
                                                ALU.add)
                        nc.vector.tensor_add(hmid[m][:, QS[q]],
                                             xs_cur[m][:, QS[q]], y)
                    mm_feature(ph, d_wout[i], o_sb, DC, ev_out, "w8b")
                otpool.release()

                with tc.tile_pool(name=f"ph_d{i}t2", bufs=1) as ph:
                    xn2 = layer_norm(
                        ph, hmid, vec, 2, 3, f"d{i}l2", out_pool=blk,
                        filler1=lambda: emit_e_w2(i, ph, range(0, 4)),
                        filler2=lambda: emit_e_w2(i, ph, range(4, DC)))

                with tc.tile_pool(name=f"ph_d{i}t3", bufs=1) as ph:
                    for qh in range(2):
                        g_half = ph.tile([P, FC, 512], BF16, tag="g_half",
                                         bufs=1, name="g")

                        def ev_g(m, q, ps, i=i, g_half=g_half):
                            act(g_half[:, m, :], ps, AF.Gelu,
                                bias=ff1b_sb[i][:, m:m + 1])
                        mm_feature(ph, d_ff1[i], xn2, FC, ev_g, "w8a",
                                   qs=(qh,))

                        g_tiles = [g_half[:, m, :] for m in range(FC)]

                        def ev_f2(m, q, ps, vec=vec, ph=ph, i=i):
                            y = ph.tile([P, 512], F32, tag="y", bufs=3,
                                        name="y")
                            nc.vector.tensor_scalar(y, ps, 0.5, vec[:, m, 7:8],
                                                    ALU.mult, ALU.add)
                            xo = ph.tile([P, 512], F32, tag="xout", bufs=3,
                                         name="xo")
                            nc.vector.tensor_add(xo, hmid[m][:, QS[q]], y)
                            nc.sync.dma_start(
                                out=res_ap(d_dst[i], m)[:, QS[q]], in_=xo)
                        mm_feature(ph, d_ff2[i], g_tiles, DC, ev_f2, "w32",
                                   qs=(qh,), rhs_half=True)

        # Schedule: e-path matmuls are carved up to fill the PE-idle LN
        # windows of the d-path — e-w1 covers d-LN1, e-w2 covers d-LN2,
        # e-LN of block i+1 is emitted mid-FF of block i.
        e_xn[0] = emit_e_ln(0, espan)
        for i in range(kk):
            q_sb = emit_d_qkv(i, qpool)   # ends with the AllGather issues
            blk = tc.alloc_tile_pool(name=f"ph_d{i}blk", bufs=1)
            otpool = tc.alloc_tile_pool(name=f"ot{i}", bufs=1)
            o_sb = emit_d_att(i, q_sb, otpool)
            emit_d_tail(i, o_sb, blk, otpool)
            blk.release()
            if i + 1 < kk:
                e_xn[i + 1] = emit_e_ln(i + 1, espan)

    nc.compile()
    return nc


# ---------------------------------------------------------------------------
# host side
# ---------------------------------------------------------------------------
_prog_cache = {}


def _pack_pvec(v):
    """[1024] -> [128, 8]: out[p, i] = v[i*128 + p]."""
    return np.ascontiguousarray(np.asarray(v, np.float32).reshape(-1, P).T)


def _bf(x):
    return np.ascontiguousarray(x).astype(ml_dtypes.bfloat16)


def kernel(**inputs):
    inp = {k: np.asarray(v) for k, v in inputs.items()}
    kk = min(int(inp["max_active_blocks"]), NBLK)
    emb = inp["embodied_input"].astype(np.float32)
    dis = inp["disembodied_input"].astype(np.float32)
    tf = inp["torsion_field"].astype(np.float32)

    if kk == 0:
        return emb.copy(), dis.copy()

    def sigmoid(x):
        return 1.0 / (1.0 + np.exp(-x))

    esc_s = sigmoid(emb[0].mean(axis=0) @ inp["esel_w"].T + inp["esel_b"])
    etop = np.argsort(-esc_s, kind="stable")[:kk]
    dsc_s = sigmoid(dis[0].mean(axis=0) @ inp["dsel_w"].T + inp["dsel_b"])
    dtop = np.argsort(-dsc_s, kind="stable")[:kk]

    if kk not in _prog_cache:
        _prog_cache[kk] = build_program(kk)
    nc = _prog_cache[kk]

    wmap = {}
    for i, idx in enumerate(etop):
        wmap[f"e{i}_w1"] = _bf(inp["e_w1"][idx].T)
        wmap[f"e{i}_w2"] = _bf(inp["e_w2"][idx].T)
    for i, idx in enumerate(dtop):
        qkv_w = inp["d_qkv_w"][idx]  # [3072, 1024]
        qkv_b = inp["d_qkv_b"][idx]
        wmap[f"d{i}_wqk"] = _bf(qkv_w[:2 * DIM].T)
        wmap[f"d{i}_wv"] = _bf(qkv_w[2 * DIM:].T)
        wmap[f"d{i}_wout"] = _bf(inp["d_out_w"][idx].T)
        wmap[f"d{i}_ff1"] = _bf(inp["d_ff1_w"][idx].T)
        wmap[f"d{i}_ff2"] = _bf(inp["d_ff2_w"][idx].T)
        wmap[f"d{i}_vec"] = np.stack([
            _pack_pvec(inp["d_ln1_s"][idx]),
            _pack_pvec(inp["d_ln1_b"][idx]),
            _pack_pvec(inp["d_ln2_s"][idx]),
            _pack_pvec(inp["d_ln2_b"][idx]),
            _pack_pvec(qkv_b[:DIM] / 16.0),
            _pack_pvec(qkv_b[DIM:2 * DIM]),
            _pack_pvec(inp["d_out_b"][idx]),
            _pack_pvec(0.5 * inp["d_ff2_b"][idx]),
        ], axis=-1)
        wmap[f"d{i}_ff1b"] = _pack_pvec(inp["d_ff1_b"][idx])
        wmap[f"d{i}_bv"] = _bf(qkv_b[2 * DIM:].reshape(1, DIM))

    in_maps = []
    for c in range(NCORES):
        b, half = c // 2, c % 2
        tsl = slice(half * TLOC, (half + 1) * TLOC)
        m = dict(wmap)
        m["x_emb"] = np.ascontiguousarray(emb[b, tsl].T)
        m["x_dis"] = np.ascontiguousarray(dis[b, tsl].T)
        esc_v = 0.3 * (1.0 + 0.1 * tf[b])
        s05_v = 0.5 * (1.0 + 0.05 * tf[b])
        m["common"] = np.stack([_pack_pvec(esc_v), _pack_pvec(s05_v)], axis=-1)
        for i, idx in enumerate(etop):
            m[f"e{i}_vec"] = np.stack([
                _pack_pvec(inp["e_ln_s"][idx]),
                _pack_pvec(inp["e_ln_b"][idx]),
                _pack_pvec(inp["e_b1"][idx]),
                _pack_pvec(esc_v * inp["e_b2"][idx]),
            ], axis=-1)
        in_maps.append(m)

    from concourse.bass_utils import run_bass_kernel_spmd
    res = run_bass_kernel_spmd(nc, in_maps, list(range(NCORES)))

    h = np.empty((B, S, DIM), np.float32)
    dh = np.empty((B, S, DIM), np.float32)
    for c in range(NCORES):
        b, half = c // 2, c % 2
        tsl = slice(half * TLOC, (half + 1) * TLOC)
        h[b, tsl] = res.results[c]["out_e"].T
        dh[b, tsl] = res.results[c]["out_d"].T
    return h, dh



# revision 47
# speedup vs baseline: 1.0961x; 1.0743x over previous
"""Trainium2 Bass kernel for nn_DevelopmentalLayerV51 (moe_routing).

kernel(**inputs) takes the FULL unsharded inputs (as reference.setup_inputs)
and returns the full (h, dh) tuple of np.float32 arrays.

Sharding: data-parallel over the B*S=8192 tokens across 8 cores (core c owns
batch c//2, sequence half c%2 -> 1024 tokens). Top-k routing uses only
scores[0], so it is decided on host; only the selected blocks' weights are
shipped (replicated, bf16). On device all activations are feature-major
[D, tokens] so every matmul is transpose-free (contraction dim in
partitions for both operands); LayerNorm/softmax partition-reductions use
ones-matmuls on the PE. Attention K/V are AllGathered within the core pair
sharing a batch; delayed = mean_S(disembodied) via a tiny pair AllReduce.
Residual streams stay fp32 and round-trip through DRAM between phases so
SBUF only ever holds one phase's working set.
"""
import sys

sys.path.insert(0, "/opt/trn_rl_repo")

import contextlib

import numpy as np
import ml_dtypes

import concourse.bass as bass
import concourse.tile as tile
from concourse import bacc, mybir

DIM = 1024
NBLK = 8
NHEAD = 4
HD = DIM // NHEAD          # 256
B, S = 4, 2048
NCORES = 8
TLOC = (B * S) // NCORES   # 1024 tokens per core
P = 128
DC = DIM // P              # 8 feature tiles
DT = TLOC // P             # 8 token tiles
FF = 4 * DIM               # 4096
FC = FF // P               # 32
F32 = mybir.dt.float32
BF16 = mybir.dt.bfloat16
AF = mybir.ActivationFunctionType
AX = mybir.AxisListType
ALU = mybir.AluOpType

PAIRS = [[0, 1], [2, 3], [4, 5], [6, 7]]
QS = [slice(0, 512), slice(512, 1024)]


def build_program(kk):
    nc = bacc.Bacc("TRN2", target_bir_lowering=False, debug=False,
                   num_devices=NCORES)
    dp = nc.declare_dram_parameter

    x_emb = dp("x_emb", [DIM, TLOC], F32, isOutput=False)
    x_dis = dp("x_dis", [DIM, TLOC], F32, isOutput=False)
    common = dp("common", [P, DC, 2], F32, isOutput=False)  # esc, s05
    e_w1 = [dp(f"e{i}_w1", [DIM, DIM], BF16, isOutput=False) for i in range(kk)]
    e_w2 = [dp(f"e{i}_w2", [DIM, DIM], BF16, isOutput=False) for i in range(kk)]
    # cols: ln_s, ln_b, b1, esc*b2
    e_vec = [dp(f"e{i}_vec", [P, DC, 4], F32, isOutput=False) for i in range(kk)]
    d_wqk = [dp(f"d{i}_wqk", [DIM, 2 * DIM], BF16, isOutput=False) for i in range(kk)]
    d_wv = [dp(f"d{i}_wv", [DIM, DIM], BF16, isOutput=False) for i in range(kk)]
    d_wout = [dp(f"d{i}_wout", [DIM, DIM], BF16, isOutput=False) for i in range(kk)]
    d_ff1 = [dp(f"d{i}_ff1", [DIM, FF], BF16, isOutput=False) for i in range(kk)]
    d_ff2 = [dp(f"d{i}_ff2", [FF, DIM], BF16, isOutput=False) for i in range(kk)]
    # cols: ln1_s, ln1_b, ln2_s, ln2_b, bq/16, bk, b_out, 0.5*b_ff2
    d_vec = [dp(f"d{i}_vec", [P, DC, 8], F32, isOutput=False) for i in range(kk)]
    d_ff1b = [dp(f"d{i}_ff1b", [P, FC], F32, isOutput=False) for i in range(kk)]
    d_bv = [dp(f"d{i}_bv", [1, DIM], BF16, isOutput=False) for i in range(kk)]
    out_e = dp("out_e", [DIM, TLOC], F32, isOutput=True)
    out_d = dp("out_d", [DIM, TLOC], F32, isOutput=True)

    dt_ = nc.dram_tensor
    cc_del_in = dt_("cc_del_in", [P, DC], F32)
    cc_del_out = dt_("cc_del_out", [P, DC], F32)
    e_mid = [[dt_(f"e_mid{i}_{m}", [P, TLOC], F32) for m in range(DC)]
             for i in range(max(kk - 1, 0))]
    d_mid = [[dt_(f"d_mid{i}_{m}", [P, TLOC], F32) for m in range(DC)]
             for i in range(max(kk - 1, 0))]
    k_loc = [dt_(f"k_loc{i}", [DIM, TLOC], BF16) for i in range(kk)]
    v_loc = [dt_(f"v_loc{i}", [DIM, TLOC], BF16) for i in range(kk)]
    k_full = [dt_(f"k_full{i}", [2, DIM, TLOC], BF16) for i in range(kk)]
    v_full = [dt_(f"v_full{i}", [2, DIM, TLOC], BF16) for i in range(kk)]

    def res_ap(t, m):
        return t[m][:, :] if isinstance(t, list) else t[m * P:(m + 1) * P, :]

    e_src = [x_emb] + e_mid
    e_dst = e_mid + [out_e]
    d_src = [x_dis] + d_mid
    d_dst = d_mid + [out_d]

    with tile.TileContext(nc, pool_alloc_mode="queue") as tc, \
         contextlib.ExitStack() as octx:
        singles = octx.enter_context(tc.tile_pool(name="singles", bufs=1))
        stats = octx.enter_context(tc.tile_pool(name="stats", bufs=1))
        qpool = octx.enter_context(tc.tile_pool(name="qp", bufs=1))
        espan = octx.enter_context(tc.tile_pool(name="esp", bufs=1))
        h1pool = octx.enter_context(tc.tile_pool(name="h1p", bufs=1))
        psum = octx.enter_context(tc.tile_pool(name="psum", bufs=8,
                                               space="PSUM"))
        e_xn = {}
        e_h1 = [[] for _ in range(kk)]

        def ps_tile(name):
            return psum.tile([P, 512], F32, tag="ps", name=name)

        def ps1_tile(name):
            return psum.tile([1, 512], F32, tag="ps", name=name)

        ones128_bf = singles.tile([P, 1], BF16)
        nc.vector.memset(ones128_bf, 1.0)
        ones1_f32 = singles.tile([1, P], F32)
        nc.vector.memset(ones1_f32, 1.0)
        ones1_bf = singles.tile([1, P], BF16)
        nc.vector.memset(ones1_bf, 1.0)

        eps_sb = singles.tile([P, 1], F32)
        nc.vector.memset(eps_sb, 1e-5)
        common_sb = singles.tile([P, DC, 2], F32)
        nc.sync.dma_start(out=common_sb, in_=common[:, :, :])
        evec_sb = []
        dvec_sb = []
        ff1b_sb = []
        bv_sb = []
        for i in range(kk):
            t = singles.tile([P, DC, 4], F32, name=f"evec{i}", tag=f"evec{i}")
            nc.sync.dma_start(out=t, in_=e_vec[i][:, :, :])
            evec_sb.append(t)
            t = singles.tile([P, DC, 8], F32, name=f"dvec{i}", tag=f"dvec{i}")
            nc.sync.dma_start(out=t, in_=d_vec[i][:, :, :])
            dvec_sb.append(t)
            t = singles.tile([P, FC], F32, name=f"ff1b{i}", tag=f"ff1b{i}")
            nc.sync.dma_start(out=t, in_=d_ff1b[i][:, :])
            ff1b_sb.append(t)
            t = singles.tile([1, DIM], BF16, name=f"bv{i}", tag=f"bv{i}")
            nc.sync.dma_start(out=t, in_=d_bv[i][:, :])
            bv_sb.append(t)

        def act(out, in_, func=AF.Copy, bias=0.0, scale=1.0):
            if func == AF.Copy and not isinstance(bias, float):
                func = AF.Identity  # Copy rejects AP bias; Identity is affine
            nc.scalar.activation(out=out, in_=in_, func=func, bias=bias,
                                 scale=scale)

        def w_cols(pool, w_dram, kc, m0, msz, tag, c0=0, csz=None):
            """[kc*128, *] bf16 DRAM weight -> sbuf [P, csz, msz] col block."""
            csz = kc if csz is None else csz
            t = pool.tile([P, csz, msz], BF16, tag=tag,
                          bufs=(4 if csz <= 8 else 2), name=f"w_{tag}")
            src = w_dram.rearrange("(k p) m -> p k m", p=P)
            nc.gpsimd.dma_start(out=t, in_=src[:, c0:c0 + csz, m0:m0 + msz])
            return t

        def mm_feature(wpool, w_dram, rhs_tiles, n_out, evict, wtag,
                       m_range=None, qs=(0, 1), rhs_half=False):
            """for m: psum[q] = W[:, mP:(m+1)P].T @ rhs[:, q-half];
            evict(m, q, ps[P,512])."""
            kc = len(rhs_tiles)
            KCH = 16
            for m in (range(n_out) if m_range is None else m_range):
                pss = {q: ps_tile(f"ps_{wtag}{m}_{q}") for q in qs}
                for c0 in range(0, kc, KCH):
                    cs = min(KCH, kc - c0)
                    wt = w_cols(wpool, w_dram, kc, m * P, P, wtag, c0, cs)
                    for d in range(cs):
                        for q in qs:
                            rhs = (rhs_tiles[c0 + d][:, 0:512] if rhs_half
                                   else rhs_tiles[c0 + d][:, QS[q]])
                            nc.tensor.matmul(
                                pss[q], wt[:, d, :], rhs,
                                start=(c0 + d == 0), stop=(c0 + d == kc - 1))
                for q in qs:
                    evict(m, q, pss[q])

        def layer_norm(ph, src, vec, s_col, b_col, name, out_pool=None,
                       xs_hook=None, filler1=None, filler2=None):
            """src: DRAM [DIM, TLOC] fp32 AP, or list of 8 sbuf fp32 tiles.
            Returns 8 bf16 [P, TLOC] normalized tiles (tag ln_xn).
            filler1/filler2: callables emitting PE work to cover the
            var/rstd chain and the per-tile normalize wave."""
            from_dram = callable(src)
            mean_ps = [ps1_tile(f"lnm_{name}{q}") for q in range(2)]
            sq_ps = [ps1_tile(f"lnsq_{name}{q}") for q in range(2)]
            xb_l = []
            for d in range(DC):
                if from_dram:
                    xsrc = ph.tile([P, TLOC], F32, tag="ln_xs", bufs=3,
                                   name=f"lnxs_{name}")
                    nc.sync.dma_start(out=xsrc, in_=src(d))
                else:
                    xsrc = src[d]
                if xs_hook is not None:
                    xs_hook(d, xsrc)
                xb = ph.tile([P, TLOC], BF16, tag="ln_xb", bufs=DC + 1,
                             name=f"lnxb_{name}")
                nc.vector.tensor_copy(xb, xsrc)
                xb_l.append(xb)
                sq = ph.tile([P, TLOC], BF16, tag="ln_sq", name=f"lnq_{name}")
                act(sq, xb, AF.Square)
                for q in range(2):
                    nc.tensor.matmul(mean_ps[q], ones128_bf, xb[:, QS[q]],
                                     start=(d == 0), stop=(d == DC - 1))
                    nc.tensor.matmul(sq_ps[q], ones128_bf, sq[:, QS[q]],
                                     start=(d == 0), stop=(d == DC - 1))
            nm = stats.tile([1, TLOC], F32, tag="st1", bufs=3, name=f"nm_{name}")
            msq = stats.tile([1, TLOC], F32, tag="st1", bufs=3,
                             name=f"msq_{name}")
            var = stats.tile([1, TLOC], F32, tag="st1", bufs=3,
                             name=f"var_{name}")
            std = stats.tile([1, TLOC], F32, tag="st1", bufs=3,
                             name=f"std_{name}")
            rstd = stats.tile([1, 2, TLOC], F32, tag="st2", name=f"rstd_{name}")
            for q in range(2):
                act(nm[:, QS[q]], mean_ps[q], AF.Copy, scale=-1.0 / DIM)
                act(msq[:, QS[q]], sq_ps[q], AF.Copy, scale=1.0 / DIM)
            act(var, nm, AF.Square)
            nc.vector.tensor_sub(var, msq, var)
            act(std, var, AF.Sqrt, bias=eps_sb[:1])
            nc.vector.reciprocal(rstd[:, 0, :], std)
            nc.vector.tensor_mul(rstd[:, 1, :], nm, rstd[:, 0, :])
            if filler1 is not None:
                filler1()
            bc = ph.tile([P, 2, TLOC], BF16, tag="ln_bc", bufs=1,
                         name=f"bc_{name}")
            for j in range(2):
                for q in range(2):
                    bc_ps = ps_tile(f"lnbc_{name}{j}{q}")
                    nc.tensor.matmul(bc_ps, ones1_f32, rstd[:, j, QS[q]],
                                     start=True, stop=True)
                    act(bc[:, j, QS[q]], bc_ps)
            if filler2 is not None:
                filler2()
            out_tiles = []
            for d in range(DC):
                xsrc = xb_l[d]
                t1 = ph.tile([P, TLOC], BF16, tag="ln_t1", bufs=4,
                             name=f"t1_{name}")
                nc.vector.tensor_mul(t1, xsrc, bc[:, 0, :])
                nc.vector.tensor_add(t1, t1, bc[:, 1, :])
                xn = (out_pool or ph).tile([P, TLOC], BF16, tag="ln_xn",
                                           bufs=DC + 1, name=f"xn_{name}")
                act(xn, t1, AF.Copy, bias=vec[:, d, b_col:b_col + 1],
                    scale=vec[:, d, s_col:s_col + 1])
                out_tiles.append(xn)
            return out_tiles

        # delayed = mean_S(x_dis): partial sums ride d0's LN1 stream, then
        # a pair AllReduce. dacc lives in singles; finalized in emit_d_qkv(0).
        delayed_sb = singles.tile([P, DC], F32)
        dacc = singles.tile([P, DC], F32, name="dacc")

        def delayed_hook(d, xsrc):
            r = stats.tile([P, 1], F32, tag="dred", bufs=3, name="dred")
            nc.vector.tensor_reduce(r, xsrc, axis=AX.X, op=ALU.add)
            nc.vector.tensor_scalar_mul(dacc[:, d:d + 1], r, 1.0 / S)

        def delayed_finalize():
            nc.sync.dma_start(out=cc_del_in[:, :], in_=dacc)
            nc.gpsimd.collective_compute(
                "AllReduce", ALU.add, replica_groups=PAIRS,
                ins=[cc_del_in[:, :]], outs=[cc_del_out[:, :]])
            nc.sync.dma_start(out=delayed_sb, in_=cc_del_out[:, :])

        def emit_e_ln(i, espan):
            vec = evec_sb[i]
            with tc.tile_pool(name=f"ph_e{i}l", bufs=1) as ph:
                return layer_norm(ph, lambda m, i=i: res_ap(e_src[i], m),
                                  vec, 0, 1, f"e{i}", out_pool=espan)

        def emit_e_w1(i, xn, wpool, m_range):
            """tanh(W1 @ xn) -> h1 tiles (persistent h1pool)."""
            vec = evec_sb[i]
            h1 = e_h1[i]

            def ev_tanh(m, q, ps, vec=vec):
                if q == 0:
                    while len(h1) <= m:
                        h1.append(h1pool.tile([P, TLOC], BF16, tag="h1",
                                              bufs=DC + 1, name="h1"))
                act(h1[m][:, QS[q]], ps, AF.Tanh, bias=vec[:, m, 2:3])
            mm_feature(wpool, e_w1[i], xn, DC, ev_tanh, "w8a",
                       m_range=m_range)

        def emit_e_w2(i, wpool, m_range):
            vec = evec_sb[i]
            xs_cur = {}

            def ev_e2(m, q, ps, vec=vec, i=i, xs_cur=xs_cur):
                if q == 0:
                    xs = wpool.tile([P, TLOC], F32, tag="xadd_e", bufs=3,
                                    name="xs")
                    nc.sync.dma_start(out=xs, in_=res_ap(e_src[i], m))
                    xs_cur[m] = xs
                y = wpool.tile([P, 512], F32, tag="y_e", bufs=3, name="y")
                act(y, ps, bias=vec[:, m, 3:4], scale=common_sb[:, m, 0:1])
                xo = wpool.tile([P, 512], F32, tag="xout_e", bufs=3, name="xo")
                nc.vector.tensor_add(xo, xs_cur[m][:, QS[q]], y)
                nc.sync.dma_start(
                    out=res_ap(e_dst[i], m)[:, QS[q]], in_=xo)
            mm_feature(wpool, e_w2[i], e_h1[i], DC, ev_e2, "w8b",
                       m_range=m_range)

        def emit_d_qkv(i, qpool):
            vec = dvec_sb[i]
            q_sb = [qpool.tile([P, TLOC], BF16, tag="q_sb", bufs=DC,
                               name=f"qsb{m}") for m in range(DC)]
            with tc.tile_pool(name=f"ph_d{i}a", bufs=1) as ph:
                emit_e_w1(i, e_xn[i], ph, range(0, 5))
                xn1 = layer_norm(
                    ph, lambda m, i=i: res_ap(d_src[i], m),
                    vec, 0, 1, f"d{i}l1",
                    xs_hook=(delayed_hook if i == 0 else None),
                    filler1=lambda: emit_e_w1(i, e_xn[i], ph, range(5, DC)))
                if i == 0:
                    delayed_finalize()

                def ev_qk(m, q, ps, vec=vec, ph=ph, i=i):
                    if m < DC:
                        act(q_sb[m][:, QS[q]], ps, bias=vec[:, m, 4:5],
                            scale=1.0 / 16.0)
                        return
                    t = ph.tile([P, 512], BF16, tag="qk_ev", bufs=4,
                                name="qkev")
                    if True:
                        act(t, ps, bias=vec[:, m - DC, 5:6])
                        nc.sync.dma_start(
                            out=k_loc[i][(m - DC) * P:(m - DC + 1) * P, QS[q]],
                            in_=t)
                mm_feature(ph, d_wqk[i], xn1, 2 * DC, ev_qk, "w8a",
                           m_range=range(DC, 2 * DC))

                nc.gpsimd.collective_compute(
                    "AllGather", ALU.bypass, replica_groups=PAIRS,
                    ins=[k_loc[i][:, :]], outs=[k_full[i][:, :, :]])

                # V token-major: lhsT = xn1 slices, rhs = Wv row-chunks
                wv_rows = []
                wv_src = d_wv[i].rearrange("(k p) m -> p k m", p=P)
                for d in range(DC):
                    t = ph.tile([P, DIM], BF16, tag="wv_row", bufs=DC,
                                name="wvr")
                    nc.gpsimd.dma_start(out=t, in_=wv_src[:, d, :])
                    wv_rows.append(t)
                for m in range(DT):
                    pss = [ps_tile(f"ps_v{m}_{q}") for q in range(2)]
                    for d in range(DC):
                        for q in range(2):
                            nc.tensor.matmul(
                                pss[q], xn1[d][:, m * P:(m + 1) * P],
                                wv_rows[d][:, QS[q]],
                                start=(d == 0), stop=False)
                    for q in range(2):
                        nc.tensor.matmul(pss[q], ones1_bf, bv_sb[i][:, QS[q]],
                                         start=False, stop=True)
                        vt = ph.tile([P, 512], BF16, tag="qk_ev", bufs=4,
                                     name="vev")
                        act(vt, pss[q])
                        nc.sync.dma_start(
                            out=v_loc[i][m * P:(m + 1) * P, QS[q]], in_=vt)

                nc.gpsimd.collective_compute(
                    "AllGather", ALU.bypass, replica_groups=PAIRS,
                    ins=[v_loc[i][:, :]], outs=[v_full[i][:, :, :]])

                mm_feature(ph, d_wqk[i], xn1, DC, ev_qk, "w8a",
                           m_range=range(DC))

            return q_sb

        def emit_d_att(i, q_sb, opool):
            o_sb = [None] * DC
            with tc.tile_pool(name=f"ph_d{i}b", bufs=1) as ph:
                pend = None

                def head_tail(h, av_sb, rden):
                    rb = ph.tile([P, TLOC], BF16, tag="rb", bufs=2, name="rb")
                    for q in range(2):
                        rb_ps = ps_tile(f"rb{h}{q}")
                        nc.tensor.matmul(rb_ps, ones1_bf, rden[:, QS[q]],
                                         start=True, stop=True)
                        act(rb[:, QS[q]], rb_ps)
                    for s2 in range(2):
                        ot = opool.tile([P, TLOC], BF16, tag="ot", bufs=DC,
                                        name="ot")
                        nc.vector.tensor_mul(ot, av_sb[s2], rb)
                        o_sb[2 * h + s2] = ot

                for h in range(NHEAD):
                    v_res = []
                    for kt in range(2 * DT):
                        r, m = kt // DT, kt % DT
                        t = ph.tile([P, HD], BF16, tag="v_res",
                                    bufs=2 * DT + 4, name="vres")
                        nc.sync.dma_start(
                            out=t, in_=v_full[i][r, m * P:(m + 1) * P,
                                                 h * HD:(h + 1) * HD])
                        v_res.append(t)
                    qh = [q_sb[2 * h + s] for s in range(2)]
                    av_ps = [[ps_tile(f"av{h}_{s2}_{q}") for q in range(2)]
                             for s2 in range(2)]
                    den_ps = [ps1_tile(f"den{h}_{q}") for q in range(2)]
                    for kt_g in range(4):
                        r, cb = kt_g // 2, (kt_g % 2) * 512
                        kTs = []
                        for s in range(2):
                            t = ph.tile([P, 512], BF16, tag="kT", bufs=4,
                                        name="kT")
                            fr = h * HD + s * P
                            nc.gpsimd.dma_start(
                                out=t,
                                in_=k_full[i][r, fr:fr + P, cb:cb + 512])
                            kTs.append(t)
                        for j in range(4):
                            kt = kt_g * 4 + j
                            first, last = kt == 0, kt == 2 * DT - 1
                            for q in range(2):
                                sc = ps_tile(f"sc{h}_{kt}_{q}")
                                for s in range(2):
                                    nc.tensor.matmul(
                                        sc, kTs[s][:, j * P:(j + 1) * P],
                                        qh[s][:, QS[q]],
                                        start=(s == 0), stop=(s == 1))
                                et = ph.tile([P, 512], BF16, tag="expT",
                                             bufs=6, name="et")
                                act(et, sc, AF.Exp)
                                nc.tensor.matmul(den_ps[q], ones128_bf, et,
                                                 start=first, stop=last)
                                for s2 in range(2):
                                    nc.tensor.matmul(
                                        av_ps[s2][q],
                                        v_res[kt][:, s2 * P:(s2 + 1) * P], et,
                                        start=first, stop=last)
                    # free PSUM fast: den -> sbuf (Scalar copy), av -> sbuf
                    # (DVE copy, unnormalized); the slow DVE reciprocal and
                    # the rb/ot tail run during the NEXT head's matmuls.
                    den_sb = stats.tile([1, TLOC], F32, tag="densb", bufs=2,
                                        name="densb")
                    rden = stats.tile([1, TLOC], BF16, tag="rden", bufs=2,
                                      name="rden")
                    for q in range(2):
                        act(den_sb[:, QS[q]], den_ps[q])
                    av_sb = [ph.tile([P, TLOC], BF16, tag="av_sb", bufs=5,
                                     name="avsb") for _ in range(2)]
                    for s2 in range(2):
                        for q in range(2):
                            nc.vector.tensor_copy(av_sb[s2][:, QS[q]],
                                                  av_ps[s2][q])
                    with nc.allow_low_precision(reason="1/den bf16 bcast"):
                        for q in range(2):
                            nc.vector.reciprocal(rden[:, QS[q]],
                                                 den_sb[:, QS[q]])
                    if pend is not None:
                        head_tail(*pend)
                    pend = (h, av_sb, rden)
                head_tail(*pend)
            return o_sb

        def emit_d_tail(i, o_sb, blk, otpool):
            vec = dvec_sb[i]
            bias_d = singles.tile([P, DC], F32, tag=f"biasd{i}",
                                  name=f"biasd{i}")
            nc.vector.tensor_scalar_mul(bias_d, delayed_sb, 0.3)
            nc.vector.tensor_add(bias_d, bias_d, vec[:, :, 6])
            nc.vector.tensor_mul(bias_d, bias_d, common_sb[:, :, 1])
            if True:
                hmid = []
                xn2 = None
                with tc.tile_pool(name=f"ph_d{i}t1", bufs=1) as ph:
                    xs_cur = {}

                    def ev_out(m, q, ps, vec=vec, ph=ph, i=i):
                        if q == 0:
                            xs = ph.tile([P, TLOC], F32, tag="xadd", bufs=3,
                                         name="xs")
                            nc.sync.dma_start(
                                out=xs, in_=res_ap(d_src[i], m))
                            xs_cur[m] = xs
                            hmid.append(blk.tile([P, TLOC], F32, tag="hmid",
                                                 bufs=DC, name="hm"))
                        y = ph.tile([P, 512], F32, tag="y", bufs=4, name="y")
                        nc.vector.tensor_scalar(y, ps, common_sb[:, m, 1:2],
                                                bias_d[:, m:m + 1], ALU.mult,
                                                ALU.add)
                        nc.vector.tensor_add(hmid[m][:, QS[q]],
                                             xs_cur[m][:, QS[q]], y)
                    mm_feature(ph, d_wout[i], o_sb, DC, ev_out, "w8b")
                otpool.release()

                with tc.tile_pool(name=f"ph_d{i}t2", bufs=1) as ph:
                    xn2 = layer_norm(
                        ph, hmid, vec, 2, 3, f"d{i}l2", out_pool=blk,
                        filler1=lambda: emit_e_w2(i, ph, range(0, 4)),
                        filler2=lambda: emit_e_w2(i, ph, range(4, DC)))

                with tc.tile_pool(name=f"ph_d{i}t3", bufs=1) as ph:
                    for qh in range(2):
                        g_half = ph.tile([P, FC, 512], BF16, tag="g_half",
                                         bufs=1, name="g")

                        def ev_g(m, q, ps, i=i, g_half=g_half):
                            act(g_half[:, m, :], ps, AF.Gelu,
                                bias=ff1b_sb[i][:, m:m + 1])
                        mm_feature(ph, d_ff1[i], xn2, FC, ev_g, "w8a",
                                   qs=(qh,))

                        g_tiles = [g_half[:, m, :] for m in range(FC)]

                        def ev_f2(m, q, ps, vec=vec, ph=ph, i=i):
                            y = ph.tile([P, 512], F32, tag="y", bufs=3,
                                        name="y")
                            act(y, ps, bias=vec[:, m, 7:8], scale=0.5)
                            xo = ph.tile([P, 512], F32, tag="xout", bufs=3,
                                         name="xo")
                            nc.vector.tensor_add(xo, hmid[m][:, QS[q]], y)
                            nc.sync.dma_start(
                                out=res_ap(d_dst[i], m)[:, QS[q]], in_=xo)
                        mm_feature(ph, d_ff2[i], g_tiles, DC, ev_f2, "w32",
                                   qs=(qh,), rhs_half=True)

        # Schedule: e-path matmuls are carved up to fill the PE-idle LN
        # windows of the d-path — e-w1 covers d-LN1, e-w2 covers d-LN2,
        # e-LN of block i+1 is emitted mid-FF of block i.
        e_xn[0] = emit_e_ln(0, espan)
        for i in range(kk):
            q_sb = emit_d_qkv(i, qpool)   # ends with the AllGather issues
            blk = tc.alloc_tile_pool(name=f"ph_d{i}blk", bufs=1)
            otpool = tc.alloc_tile_pool(name=f"ot{i}", bufs=1)
            o_sb = emit_d_att(i, q_sb, otpool)
            emit_d_tail(i, o_sb, blk, otpool)
            blk.release()
            if i + 1 < kk:
                e_xn[i + 1] = emit_e_ln(i + 1, espan)

    nc.compile()
    return nc


# ---------------------------------------------------------------------------
# host side
# ---------------------------------------------------------------------------
_prog_cache = {}


def _pack_pvec(v):
    """[1024] -> [128, 8]: out[p, i] = v[i*128 + p]."""
    return np.ascontiguousarray(np.asarray(v, np.float32).reshape(-1, P).T)


def _bf(x):
    return np.ascontiguousarray(x).astype(ml_dtypes.bfloat16)


def kernel(**inputs):
    inp = {k: np.asarray(v) for k, v in inputs.items()}
    kk = min(int(inp["max_active_blocks"]), NBLK)
    emb = inp["embodied_input"].astype(np.float32)
    dis = inp["disembodied_input"].astype(np.float32)
    tf = inp["torsion_field"].astype(np.float32)

    if kk == 0:
        return emb.copy(), dis.copy()

    def sigmoid(x):
        return 1.0 / (1.0 + np.exp(-x))

    esc_s = sigmoid(emb[0].mean(axis=0) @ inp["esel_w"].T + inp["esel_b"])
    etop = np.argsort(-esc_s, kind="stable")[:kk]
    dsc_s = sigmoid(dis[0].mean(axis=0) @ inp["dsel_w"].T + inp["dsel_b"])
    dtop = np.argsort(-dsc_s, kind="stable")[:kk]

    if kk not in _prog_cache:
        _prog_cache[kk] = build_program(kk)
    nc = _prog_cache[kk]

    wmap = {}
    for i, idx in enumerate(etop):
        wmap[f"e{i}_w1"] = _bf(inp["e_w1"][idx].T)
        wmap[f"e{i}_w2"] = _bf(inp["e_w2"][idx].T)
    for i, idx in enumerate(dtop):
        qkv_w = inp["d_qkv_w"][idx]  # [3072, 1024]
        qkv_b = inp["d_qkv_b"][idx]
        wmap[f"d{i}_wqk"] = _bf(qkv_w[:2 * DIM].T)
        wmap[f"d{i}_wv"] = _bf(qkv_w[2 * DIM:].T)
        wmap[f"d{i}_wout"] = _bf(inp["d_out_w"][idx].T)
        wmap[f"d{i}_ff1"] = _bf(inp["d_ff1_w"][idx].T)
        wmap[f"d{i}_ff2"] = _bf(inp["d_ff2_w"][idx].T)
        wmap[f"d{i}_vec"] = np.stack([
            _pack_pvec(inp["d_ln1_s"][idx]),
            _pack_pvec(inp["d_ln1_b"][idx]),
            _pack_pvec(inp["d_ln2_s"][idx]),
            _pack_pvec(inp["d_ln2_b"][idx]),
            _pack_pvec(qkv_b[:DIM] / 16.0),
            _pack_pvec(qkv_b[DIM:2 * DIM]),
            _pack_pvec(inp["d_out_b"][idx]),
            _pack_pvec(0.5 * inp["d_ff2_b"][idx]),
        ], axis=-1)
        wmap[f"d{i}_ff1b"] = _pack_pvec(inp["d_ff1_b"][idx])
        wmap[f"d{i}_bv"] = _bf(qkv_b[2 * DIM:].reshape(1, DIM))

    in_maps = []
    for c in range(NCORES):
        b, half = c // 2, c % 2
        tsl = slice(half * TLOC, (half + 1) * TLOC)
        m = dict(wmap)
        m["x_emb"] = np.ascontiguousarray(emb[b, tsl].T)
        m["x_dis"] = np.ascontiguousarray(dis[b, tsl].T)
        esc_v = 0.3 * (1.0 + 0.1 * tf[b])
        s05_v = 0.5 * (1.0 + 0.05 * tf[b])
        m["common"] = np.stack([_pack_pvec(esc_v), _pack_pvec(s05_v)], axis=-1)
        for i, idx in enumerate(etop):
            m[f"e{i}_vec"] = np.stack([
                _pack_pvec(inp["e_ln_s"][idx]),
                _pack_pvec(inp["e_ln_b"][idx]),
                _pack_pvec(inp["e_b1"][idx]),
                _pack_pvec(esc_v * inp["e_b2"][idx]),
            ], axis=-1)
        in_maps.append(m)

    from concourse.bass_utils import run_bass_kernel_spmd
    res = run_bass_kernel_spmd(nc, in_maps, list(range(NCORES)))

    h = np.empty((B, S, DIM), np.float32)
    dh = np.empty((B, S, DIM), np.float32)
    for c in range(NCORES):
        b, half = c // 2, c % 2
        tsl = slice(half * TLOC, (half + 1) * TLOC)
        h[b, tsl] = res.results[c]["out_e"].T
        dh[b, tsl] = res.results[c]["out_d"].T
    return h, dh

